# revision 29
# baseline (speedup 1.0000x reference)
"""Sigmoid-attention kernel for Trainium2, SPMD over 8 NeuronCores.

Reference computation (per batch b, head h):
    q = (x @ Wq_h) * SCALE ; k = x @ Wk_h ; v = x[:, :, h*64:(h+1)*64]
    out_h = sigmoid((q + bias_h) @ k^T) @ v
Sharding: 8 cores = 4 batches x 2 head-groups (4 heads each).
Each core computes its 4 heads independently; no collectives.

Heads are processed in pairs packed into the two 64-partition halves of
the PE array: head A lives on SBUF partitions 0-63, head B on 64-127.
Scores run as two concurrent 64x128 row-tiles; the P@V matmuls run as
two concurrent 128x64 column-tiles writing the two PSUM halves. All
tile positions are auto-derived from AP base partitions.

All matmuls contract along SBUF partitions, so x arrives pre-transposed
(features on partitions) and the kernel computes q^T/k^T/S^T/out^T;
the host re-transposes the [dk, n] outputs into the reference layout.
"""
import sys

import numpy as np
import ml_dtypes

try:
    import concourse.bass as bass  # noqa: F401
except ImportError:
    sys.path.insert(0, "/opt/trn_rl_repo")
import concourse.tile as tile
from concourse import bacc, mybir
from concourse.bass_utils import run_bass_kernel_spmd
from concourse.dve_spec import Spec, Src0, One, C0, C1, Bin, AluOp, lower
from concourse.dve_spec import _has_src1 as _has_src1_fn
from concourse.dve_uop import DveOpSpec
from concourse.dve_ops import (
    DveOp, OPS, CUSTOM_DVE_SPECS, _SUB_OPCODE_FOR_NAME, _CUSTOM_DVE_ROW_BASE,
)

BF16 = mybir.dt.bfloat16
F32 = mybir.dt.float32
I32 = mybir.dt.int32
bf16 = ml_dtypes.bfloat16

B, N, DIM = 4, 2048, 512
HEADS, DK = 8, 64
SCALE = DK ** -0.5
NCORES = 8
HPG = 4            # heads per group (= per core)
NPAIR = HPG // 2   # head pairs per core
GD = HPG * DK      # 256: group feature width
DC = DIM // 128    # 4 d-chunks (contraction tiles for projections)
NIC = N // 512     # 4 i-chunks
NJ = N // 128      # 16 j-tiles

ACT = mybir.ActivationFunctionType
ALU = mybir.AluOpType

# Schraudolph exp constants (exp(-s) ~= bitcast_f32(int32(B - A*s)))
EXP_A = float(2 ** 23 / np.log(2.0))
EXP_B = float(1064867216)

# Custom fused DVE op: out = 1/(1 + in0) via bit-trick reciprocal seed
# (exponent flip: x*bitcast(~x) lands in [-4.5, -4]) + one Newton step.
RC0 = -0.23569351
RC1 = 2.0034004
_SIG_NAME = "SIGMOID_RECIP_TAIL_ANT"


def _ref_sig_tail(in0, in1, c0, c1, c2):
    t = (1.0 + in0).astype(np.float32)
    nt = (~t.view(np.int32)).view(np.float32)
    y0 = (nt * np.float32(c0)).astype(np.float32)
    return y0 * (np.float32(c1) - t * y0)


def _register_sig_tail():
    if _SIG_NAME in _SUB_OPCODE_FOR_NAME:
        return next(o for o in OPS if o.name == _SIG_NAME)
    t = One + Src0
    y0 = Bin(AluOp.BITWISE_NOT, t, t) * C0
    spec = Spec(body=y0 * (C1 - t * y0), reference=_ref_sig_tail)
    opcode = _CUSTOM_DVE_ROW_BASE + len(OPS)
    assert opcode < 0x20
    _SUB_OPCODE_FOR_NAME[_SIG_NAME] = opcode
    shas = {}
    for ver in ("v3", "v4"):
        try:
            sl = DveOpSpec(name=_SIG_NAME, opcode=opcode,
                           uops=lower(spec, ver=ver), rd1_en=_has_src1_fn(spec))
            shas[ver] = sl.sha(ver)
        except Exception:
            pass
    op = DveOp(_SIG_NAME, spec, subdim=False, uops_sha=shas)
    OPS.append(op)
    CUSTOM_DVE_SPECS[_SIG_NAME] = spec
    return op


SIG_OP = _register_sig_tail()


def _build():
    nc = bacc.Bacc("TRN2", target_bir_lowering=False, debug=False)
    xT = nc.declare_dram_parameter("xT", [DIM, N], BF16, isOutput=False)
    wq = nc.declare_dram_parameter("wq", [DIM, GD], BF16, isOutput=False)
    wk = nc.declare_dram_parameter("wk", [DIM, GD], BF16, isOutput=False)
    vv = nc.declare_dram_parameter("v", [N, GD], BF16, isOutput=False)
    bias = nc.declare_dram_parameter("bias", [GD, 1], F32, isOutput=False)
    out = nc.declare_dram_parameter("out", [NPAIR, 128, N], F32, isOutput=True)

    with tile.TileContext(nc) as tc:
        with (
            tc.tile_pool(name="const", bufs=1) as cpool,
            tc.tile_pool(name="qk", bufs=8) as qkpool,
            tc.tile_pool(name="pp", bufs=6) as ppool,
            tc.tile_pool(name="dve", bufs=2) as dvepool,
            tc.tile_pool(name="osb", bufs=2) as opool,
            tc.tile_pool(name="ps_proj", bufs=1, space="PSUM") as pjpool,
            tc.tile_pool(name="ps_s", bufs=2, space="PSUM") as spool,
            tc.tile_pool(name="ps_sd", bufs=1, space="PSUM") as sdpool,
            tc.tile_pool(name="ps_o", bufs=1, space="PSUM") as oppool,
        ):
            # ---- constants (bias/weights first; v on the scalar DGE queue
            #      so it streams in parallel with the xT loads) ----
            bias_t = []
            for p in range(NPAIR):
                t = cpool.tile([128, 1], F32, name=f"bias{p}")
                nc.sync.dma_start(t[:], bias[p * 128:(p + 1) * 128, :])
                bias_t.append(t)
            wq_t, wk_t = [], []
            for dc in range(DC):
                t = cpool.tile([128, GD], BF16, name=f"wqt{dc}")
                nc.sync.dma_start(t[:], wq[dc * 128:(dc + 1) * 128, :])
                wq_t.append(t)
                t = cpool.tile([128, GD], BF16, name=f"wkt{dc}")
                nc.sync.dma_start(t[:], wk[dc * 128:(dc + 1) * 128, :])
                wk_t.append(t)
            # per-(dc, ic) xT tiles so the first projection can start after
            # only 512 KB has landed (deps are tile-granular)
            xt_t = {}
            for ic in range(NIC):
                for dc in range(DC):
                    t = cpool.tile([128, 512], BF16, name=f"xt{dc}_{ic}")
                    nc.sync.dma_start(
                        t[:], xT[dc * 128:(dc + 1) * 128, ic * 512:(ic + 1) * 512])
                    xt_t[(dc, ic)] = t
            # v rearranged so partition p holds v[jc*128+p, :] for each j-chunk jc
            v_t = cpool.tile([128, NJ * GD], BF16, name="vt")
            nc.scalar.dma_start(
                v_t[:].rearrange("p (jc e) -> p jc e", jc=NJ),
                vv.rearrange("(jc p) e -> p jc e", p=128),
            )
            # pq/pk share one PSUM slot (q then k serialized; proj runs in
            # the PE slack of the attention windows)
            qbT_t, kT_t = {}, {}

            def proj_emit(p, ic):
                qbT = qkpool.tile([128, 512], BF16, tag="qbT", name=f"qbT{p}_{ic}")
                kT = qkpool.tile([128, 512], BF16, tag="kT", name=f"kT{p}_{ic}")
                pq = pjpool.tile([128, 512], F32, tag="pj", name=f"pq{p}_{ic}")
                pk = pjpool.tile([128, 512], F32, tag="pj", name=f"pk{p}_{ic}")
                ws = slice(p * 128, (p + 1) * 128)
                for dc in range(DC):
                    nc.tensor.matmul(
                        pq[:], wq_t[dc][:, ws], xt_t[(dc, ic)][:],
                        start=(dc == 0), stop=(dc == DC - 1),
                    )
                for dc in range(DC):
                    nc.tensor.matmul(
                        pk[:], wk_t[dc][:, ws], xt_t[(dc, ic)][:],
                        start=(dc == 0), stop=(dc == DC - 1),
                    )
                # qb = q*SCALE + bias (per-partition), cast to bf16 (DVE)
                nc.vector.tensor_scalar(qbT[:], pq[:], float(SCALE),
                                        bias_t[p][:, :], ALU.mult, ALU.add)
                nc.vector.tensor_copy(kT[:], pk[:])
                qbT_t[(p, ic)] = qbT
                kT_t[(p, ic)] = kT

            def scores(p, j, ic, s_ps):
                kslc = kT_t[(p, j // 4)][:, (j % 4) * 128:(j % 4 + 1) * 128]
                qslc = qbT_t[(p, ic)]
                # two concurrent 64x128 row-tiles (head A rows 0-63, B 64-127)
                nc.tensor.matmul(
                    s_ps[:, 0:512], kslc[0:64, :], qslc[0:64, :],
                    start=True, stop=True,
                )
                nc.tensor.matmul(
                    s_ps[:, 512:1024], kslc[64:128, :], qslc[64:128, :],
                    start=True, stop=True,
                )

            def pv(p, j, o_ps, p_sb, start, stop):
                ha, hb = 2 * p, 2 * p + 1
                # P @ v: two concurrent 128x64 col-tiles into PSUM halves
                nc.tensor.matmul(
                    o_ps[0:64, :],
                    v_t[:, j * GD + ha * DK: j * GD + (ha + 1) * DK],
                    p_sb[:, 0:512],
                    start=start, stop=stop,
                )
                nc.tensor.matmul(
                    o_ps[64:128, :],
                    v_t[:, j * GD + hb * DK: j * GD + (hb + 1) * DK],
                    p_sb[:, 512:1024],
                    start=start, stop=stop,
                )

            def launch_dve(p, j, ic):
                # sigmoid = 1/(1 + schraudolph_exp(-s)) on VectorE; its own
                # PSUM tile so it never blocks the ScalarE lane's buffers
                s_ps = sdpool.tile([128, 1024], F32, tag="sd",
                                   name=f"sd{p}_{ic}_{j}")
                scores(p, j, ic, s_ps)
                it = dvepool.tile([128, 1024], I32, tag="sit",
                                  name=f"sit{p}_{ic}_{j}")
                nc.vector.tensor_scalar(it[:], s_ps[:], -EXP_A, EXP_B,
                                        ALU.mult, ALU.add)
                p_sb = ppool.tile([128, 1024], BF16, tag="pgd",
                                  name=f"prd{p}_{ic}_{j}")
                nc.vector._custom_dve(SIG_OP, out=p_sb[:],
                                      in0=it[:].bitcast(F32), s0=RC0, s1=RC1)
                return p_sb

            NDVE = 4  # base j-groups per window evaluated on VectorE (4 or 5)
            for ic in range(NIC):
                proj_emit(0, ic)
            out_sbs = {p: opool.tile([128, N], F32, tag="osb", name=f"osb{p}")
                       for p in range(NPAIR)}
            pending = None  # deferred tail of the previous window

            for p in range(NPAIR):
                for ic in range(NIC):
                    # alternate 5/4: DVE-heavy first window (VectorE starts
                    # before ScalarE), ACT-light last (shorter DVE tail)
                    ndve = NDVE + ((p * NIC + ic + 1) % 2)
                    o_ps = oppool.tile([128, 512], F32, tag="ops", name=f"ops{p}_{ic}")
                    dve_p = {}
                    dve_p[NJ - ndve] = launch_dve(p, NJ - ndve, ic)
                    for j in range(NJ - ndve):
                        s_ps = spool.tile([128, 1024], F32, tag="sg",
                                          name=f"s{p}_{ic}_{j}")
                        scores(p, j, ic, s_ps)
                        p_sb = ppool.tile([128, 1024], BF16, tag="pg",
                                          name=f"pr{p}_{ic}_{j}")
                        nc.scalar.activation(p_sb[:], s_ps[:], ACT.Sigmoid)
                        if j == 0 and pending is not None:
                            # flush the previous window's tail now that this
                            # window's first sigmoid is already in flight
                            pending()
                            pending = None
                        pv(p, j, o_ps, p_sb, start=(j == 0), stop=False)
                        for step, jd in enumerate(range(NJ - ndve + 1, NJ)):
                            if j == 2 + 2 * step:
                                dve_p[jd] = launch_dve(p, jd, ic)
                        if j == 5 and p + 1 < NPAIR:
                            proj_emit(p + 1, ic)  # next pair's proj, spread out

                    def tail(p=p, ic=ic, ndve=ndve, o_ps=o_ps, dve_p=dve_p):
                        for j in range(NJ - ndve, NJ):
                            pv(p, j, o_ps, dve_p[j], start=False,
                               stop=(j == NJ - 1))
                        out_sb = out_sbs[p]
                        nc.vector.tensor_copy(
                            out_sb[:, ic * 512:(ic + 1) * 512], o_ps[:])
                        nc.sync.dma_start(out[p][:, ic * 512:(ic + 1) * 512],
                                          out_sb[:, ic * 512:(ic + 1) * 512])
                    pending = tail
            pending()
    nc.compile()
    return nc


_NC_CACHE = None


def _get_nc():
    global _NC_CACHE
    if _NC_CACHE is None:
        _NC_CACHE = _build()
    return _NC_CACHE


def _make_in_maps(x, Wq, Wk, rb):
    xT_b = [np.ascontiguousarray(x[b].T).astype(bf16) for b in range(B)]
    wq_bf = Wq.astype(bf16)
    wk_bf = Wk.astype(bf16)
    bias_flat = rb.reshape(HEADS * DK, 1)  # [512, 1] head-major

    in_maps = []
    for c in range(NCORES):
        b, g = divmod(c, 2)
        gs = slice(g * GD, (g + 1) * GD)
        in_maps.append({
            "xT": xT_b[b],
            "wq": np.ascontiguousarray(wq_bf[:, gs]),
            "wk": np.ascontiguousarray(wk_bf[:, gs]),
            "v": np.ascontiguousarray(x[b, :, gs]).astype(bf16),
            "bias": np.ascontiguousarray(bias_flat[g * GD:(g + 1) * GD]),
        })
    return in_maps


def _gather(results):
    out_full = np.empty((B, N, DIM), dtype=np.float32)
    for c in range(NCORES):
        b, g = divmod(c, 2)
        oc = results[c]["out"]  # [NPAIR, 128, N]
        for p in range(NPAIR):
            for u in range(2):
                h = 2 * p + u
                col = g * GD + h * DK
                out_full[b, :, col:col + DK] = oc[p, u * 64:(u + 1) * 64, :].T
    return out_full


def kernel(x, Wq, Wk, rel_content_bias):
    x = np.asarray(x, dtype=np.float32)
    Wq = np.asarray(Wq, dtype=np.float32)
    Wk = np.asarray(Wk, dtype=np.float32)
    rb = np.asarray(rel_content_bias, dtype=np.float32)

    nc = _get_nc()
    in_maps = _make_in_maps(x, Wq, Wk, rb)
    res = run_bass_kernel_spmd(nc, in_maps, core_ids=list(range(NCORES)))
    return _gather(res.results)
